# revision 1
# baseline (speedup 1.0000x reference)
"""Trainium2 Bass kernel for the fused candidate-attention module.

Computation (reference, fp32):
    delta[n,l,m] = sum_k self_delta[n,m,l,k]
    out[n,l]     = sum_m value_w[m] * delta[n,l,m] * (emb[1+l,:] . self_attn[n,m,:])

Sharding: candidates (L=8192) are split 1024 per core across 8 cores; every
core processes all 32 batches.  Per-core device pipeline, per batch n:

    dd   [m=100, 2048]      <- DMA of self_delta[n, :, l0:l0+1024, :]
                               (two batches per 1.6 MB DMA, alternating between
                               the SP and ACT HWDGE rings)
    ds   [m=100, 1024]      = dd[:, 0::2] + dd[:, 1::2]                 (k-sum)
    w    [m=100, d=128]     = value_w[m] * self_attn[n]
    g    [d=128, l=512]     = w^T @ ds                                  (PE)
    prod [d=128, l=512]     = g * embT                                  (DVE)
    row  [1,     l=512]     = ones^T @ prod  (partition reduction, PE)
    orow_all[0, n*1024+...] <- ACT copy; one 128 KB output DMA at the end

Modes: "f32r" (default) rounds the matmul operands to float32r on the
producing ops, which runs the PE at full rate with ~tf32 precision; "f32"
uses plain fp32 matmuls for the m-contraction and a bf16 final d-dot;
"bf16" casts delta to bf16 during DMA and folds the k-sum into two
PSUM-accumulating matmuls.
"""

import os
from contextlib import ExitStack

import numpy as np

import concourse.bacc as bacc
import concourse.bass as bass
import concourse.mybir as mybir
from concourse.bass_utils import run_bass_kernel_spmd
from concourse.masks import make_identity
from concourse.tile import TileContext

N, M, L, K, D = 32, 100, 8192, 2, 128
NCORES = 8
LC = L // NCORES  # candidates per core
MMF = 512  # matmul moving free dim (one PSUM bank of fp32)
NCHUNK = LC // MMF
NGRP = int(os.environ.get("KERNEL_NGRP", "1"))  # batches per delta DMA
DDBUFS = int(os.environ.get("KERNEL_DDBUFS", "4"))

F32 = mybir.dt.float32
F32R = mybir.dt.float32r
BF16 = mybir.dt.bfloat16
F16 = mybir.dt.float16

MODE = os.environ.get("KERNEL_MODE", "f16")
KSUM_ENGINE = os.environ.get("KERNEL_KSUM_ENGINE", "gpsimd")  # or "vector"
# Benchmarking only: device-side repeat of the main loop (1 = no loop)
LOOP_R = int(os.environ.get("KERNEL_LOOP", "1"))
# Benchmarking only: pipeline prefix to build ("dma", "ksum", "mm", "mul", "full")
STAGE = os.environ.get("KERNEL_STAGE", "full")


def _build_nc() -> bass.Bass:
    if MODE == "f32r":
        mm_dt, dd_dt, red_dt = F32R, F32, F32R
    elif MODE == "f32":
        mm_dt, dd_dt, red_dt = F32, F32, BF16
    elif MODE == "f16":
        mm_dt, dd_dt, red_dt = F16, F16, F16
    else:
        mm_dt, dd_dt, red_dt = BF16, BF16, BF16

    nc = bacc.Bacc()

    delta = nc.declare_dram_parameter("delta", [N, M, LC, K], F32, isOutput=False)
    attn = nc.declare_dram_parameter("attn", [N, M, D], F32, isOutput=False)
    emb = nc.declare_dram_parameter("emb", [LC, D], F32, isOutput=False)
    vw = nc.declare_dram_parameter("vw", [M, 1], F32, isOutput=False)
    out = nc.declare_dram_parameter("out", [N, LC], F32, isOutput=True)

    with TileContext(nc) as tc, ExitStack() as ctx:
        const = ctx.enter_context(tc.tile_pool(name="const", bufs=1))

        vw_sb = const.tile([M, 1], F32)
        nc.sync.dma_start(out=vw_sb[:], in_=vw[:])

        # self_attn, all batches: [m, n, d] so each batch is a [100, 128] slice
        attn_sb = const.tile([M, N * D], F32)
        nc.sync.dma_start(
            out=attn_sb[:].rearrange("m (n d) -> m n d", d=D),
            in_=attn[:].rearrange("n m d -> m n d"),
        )

        # emb slice in [l%128, l//128, d] layout, then transpose to [d, l]
        emb_lp = const.tile([128, (LC // 128) * D], F32)
        nc.sync.dma_start(
            out=emb_lp[:].rearrange("p (c d) -> p c d", d=D),
            in_=emb[:].rearrange("(c p) d -> p c d", p=128),
        )
        emb_dt = F16 if MODE == "f16" else F32
        embT = const.tile([D, LC], emb_dt)
        ident = const.tile([128, 128], F32)
        make_identity(nc, ident[:])
        with tc.tile_pool(name="tp_psum", bufs=2, space="PSUM") as tp_psum:
            for c in range(LC // 128):
                tp = tp_psum.tile([128, 128], F32)
                nc.tensor.transpose(tp[:], emb_lp[:, c * 128 : (c + 1) * 128], ident[:])
                nc.scalar.copy(embT[:, c * 128 : (c + 1) * 128], tp[:])

        # two identical columns: fp32r matmuls need even innermost free counts
        ones_sb = const.tile([D, 2], red_dt)
        if red_dt == F32R:
            ones_f32 = const.tile([D, 2], F32)
            nc.vector.memset(ones_f32[:], 1.0)
            nc.vector.tensor_copy(ones_sb[:], ones_f32[:])
        else:
            nc.vector.memset(ones_sb[:], 1.0)

        # output staging: col (n*LC/128 + c) holds out[n, c*128 : (c+1)*128]
        NCOL = LC // 128  # 128-wide output chunks per batch
        out_sb = const.tile([128, N * NCOL], F32)

        dd_pool = ctx.enter_context(tc.tile_pool(name="dd", bufs=DDBUFS))
        ds_pool = ctx.enter_context(tc.tile_pool(name="ds", bufs=3))
        w_pool = ctx.enter_context(tc.tile_pool(name="w", bufs=3))
        g_psum = ctx.enter_context(tc.tile_pool(name="g", bufs=3, space="PSUM"))
        gs_pool = ctx.enter_context(tc.tile_pool(name="gs", bufs=3))
        prod_pool = ctx.enter_context(tc.tile_pool(name="prod", bufs=3))
        row_psum = ctx.enter_context(tc.tile_pool(name="row", bufs=2, space="PSUM"))
        otp_psum = ctx.enter_context(tc.tile_pool(name="otp", bufs=2, space="PSUM"))
        outT_pool = ctx.enter_context(tc.tile_pool(name="outT", bufs=2))

        loop_ctx = tc.For_i(0, LOOP_R, 1) if LOOP_R > 1 else None
        if loop_ctx is not None:
            ctx.enter_context(loop_ctx)
        for grp in range(N // NGRP):
            dd = dd_pool.tile([M, NGRP * LC * K], dd_dt)
            dma_eng = nc.sync if grp % 2 == 0 else nc.scalar
            if dd_dt != F32:
                dma_eng = nc.gpsimd  # dtype-casting DMA needs SWDGE
            dma_eng.dma_start(
                out=dd[:].rearrange("m (n f) -> m n f", n=NGRP),
                in_=delta[grp * NGRP : (grp + 1) * NGRP].rearrange(
                    "n m l k -> m n (l k)"
                ),
            )
            for j in range(NGRP):
                n = grp * NGRP + j
                ddv = dd[:].rearrange("m (n l k) -> m n l k", n=NGRP, k=K)[:, j]
                if STAGE == "dma":
                    continue

                w_t = w_pool.tile([M, D], mm_dt)
                nc.vector.tensor_scalar(
                    out=w_t[:],
                    in0=attn_sb[:, n * D : (n + 1) * D],
                    scalar1=vw_sb[:, 0:1],
                    scalar2=None,
                    op0=mybir.AluOpType.mult,
                )

                if MODE in ("f32", "f32r"):
                    ds = ds_pool.tile([M, LC], mm_dt)
                    eng = nc.gpsimd if KSUM_ENGINE == "gpsimd" else nc.vector
                    eng.tensor_add(ds[:], ddv[:, :, 0], ddv[:, :, 1])

                if STAGE == "ksum":
                    continue
                row_ps = None
                if STAGE in ("full", "red"):
                    row_ps = row_psum.tile([128, 2 * NCOL], F32, tag="row_ps")
                for h in range(NCHUNK):
                    lsl = slice(h * MMF, (h + 1) * MMF)
                    g = g_psum.tile([D, MMF], F32)
                    if MODE in ("f32", "f32r"):
                        nc.tensor.matmul(
                            g[:], lhsT=w_t[:], rhs=ds[:, lsl], start=True, stop=True
                        )
                    else:
                        nc.tensor.matmul(
                            g[:], lhsT=w_t[:], rhs=ddv[:, lsl, 0],
                            start=True, stop=False,
                        )
                        nc.tensor.matmul(
                            g[:], lhsT=w_t[:], rhs=ddv[:, lsl, 1],
                            start=False, stop=True,
                        )

                    if STAGE == "mm":
                        continue
                    prod = prod_pool.tile([D, MMF], red_dt)
                    if MODE == "f16":
                        # ACT evicts PSUM as fp16 so the DVE multiply runs in
                        # its 2x packed mode (PSUM-source TT is stuck at 1x)
                        gs = gs_pool.tile([D, MMF], F16)
                        nc.scalar.copy(gs[:], g[:])
                        nc.vector.tensor_mul(prod[:], gs[:], embT[:, lsl])
                    else:
                        nc.vector.tensor_mul(prod[:], g[:], embT[:, lsl])

                    if STAGE == "mul" or row_ps is None:
                        continue
                    # partition-reduce over d: prod chunk as stationary weights,
                    # ones as the (single-column) moving operand
                    for s in range(MMF // 128):
                        c = h * (MMF // 128) + s
                        nc.tensor.matmul(
                            row_ps[:, 2 * c : 2 * c + 2],
                            lhsT=prod[:, s * 128 : (s + 1) * 128],
                            rhs=ones_sb[:],
                            start=True,
                            stop=True,
                        )

                if STAGE == "full" and row_ps is not None:
                    rview = row_ps[:].rearrange("p (c two) -> p c two", two=2)
                    nc.scalar.copy(
                        out_sb[:, n * NCOL : (n + 1) * NCOL], rview[:, :, 0]
                    )

        if STAGE == "full":
            # out flat is [(n c), p]-contiguous: transpose each 128-col block of
            # out_sb so the store DMA reads 512B per partition
            out_rows = out[:].rearrange("n (c p) -> (n c) p", p=128)
            for t in range((N * NCOL) // 128):
                tpo = otp_psum.tile([128, 128], F32)
                nc.tensor.transpose(
                    tpo[:], out_sb[:, t * 128 : (t + 1) * 128], ident[:]
                )
                outT = outT_pool.tile([128, 128], F32)
                nc.scalar.copy(outT[:], tpo[:])
                nc.sync.dma_start(
                    out=out_rows[t * 128 : (t + 1) * 128, :], in_=outT[:]
                )

    nc.compile()
    return nc


_NC_CACHE: dict[str, bass.Bass] = {}


def _get_nc() -> bass.Bass:
    key = f"{MODE}:{KSUM_ENGINE}:{LOOP_R}:{STAGE}:{NGRP}:{DDBUFS}"
    if key not in _NC_CACHE:
        _NC_CACHE[key] = _build_nc()
    return _NC_CACHE[key]


def kernel(self_attn, self_delta, emb_table, value_w, traj_len=None, loc_max=None,
           _trace=False, _tmpdir=None):
    """Full inputs in, full output out.  traj_len is unused by the reference."""
    self_attn = np.ascontiguousarray(np.asarray(self_attn, dtype=np.float32))
    self_delta = np.asarray(self_delta, dtype=np.float32)
    emb_table = np.asarray(emb_table, dtype=np.float32)
    value_w = np.ascontiguousarray(
        np.asarray(value_w, dtype=np.float32).reshape(M, 1)
    )
    assert self_attn.shape == (N, M, D), self_attn.shape
    assert self_delta.shape == (N, M, L, K), self_delta.shape
    assert emb_table.shape == (L + 1, D), emb_table.shape
    if loc_max is not None:
        assert int(loc_max) == L, loc_max

    in_maps = []
    for c in range(NCORES):
        l0 = c * LC
        in_maps.append(
            {
                "delta": np.ascontiguousarray(self_delta[:, :, l0 : l0 + LC, :]),
                "attn": self_attn,
                "emb": np.ascontiguousarray(emb_table[1 + l0 : 1 + l0 + LC]),
                "vw": value_w,
            }
        )

    nc = _get_nc()
    try:
        res = run_bass_kernel_spmd(
            nc, in_maps, list(range(NCORES)), trace=_trace, tmpdir=_tmpdir
        )
    except Exception:
        # one retry for transient NRT execution failures
        res = run_bass_kernel_spmd(
            nc, in_maps, list(range(NCORES)), trace=_trace, tmpdir=_tmpdir
        )
    out = np.concatenate([res.results[c]["out"] for c in range(NCORES)], axis=1)
    if _trace:
        return out, res
    return out

